# revision 3
# baseline (speedup 1.0000x reference)
"""Segment-mean (MeanAggregator) Trainium2 kernel.

Strategy: segment ids are sorted, so shard *segments* evenly across the 8
cores and snap edge ranges to segment boundaries on the host -> each core
owns a disjoint row range of the output and no cross-core reduction is
needed at all.

Per core, segments are processed in uniform windows of 128 segments that
live on the PSUM partition dim as a [128 segs, 65] accumulator (64 value
sums + 1 count).  Edges are consumed in chunks of 128 (the contraction
dim): a one-hot matrix O[edge, seg] = (loc[edge] == seg) is built on the
vector engine from a constant iota row, and a single TensorE matmul
  psum[seg, :] += O.T @ [values | ones]
scatter-adds the whole chunk.  Host-side preprocessing packs values/locs
into a padded uniform layout (so the SPMD program is identical on every
core) with loc = -1 on padding slots, whose one-hot row is all zero and
therefore contributes nothing.
"""

import math
import os

import numpy as np

import concourse.bacc as bacc
import concourse.mybir as mybir
import concourse.tile as tile
from concourse.bass_utils import run_bass_kernel_spmd

N_CORES = 8
P = 128      # edges per chunk == contraction dim
W_SEGS = 128  # segments per PSUM window (partition dim of the accumulator)
D = 64

_PROGRAM_CACHE = {}


def _build_program(n_win: int, n_c: int):
    key = (n_win, n_c)
    if key in _PROGRAM_CACHE:
        return _PROGRAM_CACHE[key]
    f32 = mybir.dt.float32
    nc = bacc.Bacc(
        "TRN2", target_bir_lowering=False, debug=False, num_devices=N_CORES
    )
    vals = nc.dram_tensor(
        "vals", [n_win, P, n_c * (D + 1)], f32, kind="ExternalInput"
    )
    locs = nc.dram_tensor("locs", [n_win, P, n_c], f32, kind="ExternalInput")
    outp = nc.dram_tensor("outp", [n_win * W_SEGS, D], f32, kind="ExternalOutput")

    with tile.TileContext(nc) as tc:
        with (
            tc.tile_pool(name="const", bufs=1) as cpool,
            tc.tile_pool(name="vals", bufs=3) as vpool,
            tc.tile_pool(name="locs", bufs=3) as lpool,
            tc.tile_pool(name="oh", bufs=4) as opool,
            tc.tile_pool(name="acc", bufs=2, space="PSUM") as ppool,
            tc.tile_pool(name="res", bufs=3) as rpool,
        ):
            iota = cpool.tile([P, W_SEGS], f32)
            nc.gpsimd.iota(
                iota[:],
                pattern=[[1, W_SEGS]],
                base=0,
                channel_multiplier=0,
                allow_small_or_imprecise_dtypes=True,
            )
            for w in range(n_win):
                vt = vpool.tile([P, n_c * (D + 1)], f32)
                nc.sync.dma_start(out=vt[:], in_=vals[w])
                vt3 = vt[:].rearrange("p (c e) -> p c e", e=D + 1)
                lt = lpool.tile([P, n_c], f32)
                nc.sync.dma_start(out=lt[:], in_=locs[w])
                ps = ppool.tile([W_SEGS, D + 1], f32)
                for c in range(n_c):
                    oh = opool.tile([P, W_SEGS], f32)
                    nc.vector.tensor_scalar(
                        out=oh[:],
                        in0=iota[:],
                        scalar1=lt[:, c : c + 1],
                        scalar2=None,
                        op0=mybir.AluOpType.is_equal,
                    )
                    nc.tensor.matmul(
                        out=ps[:],
                        lhsT=oh[:],
                        rhs=vt3[:, c, :],
                        start=(c == 0),
                        stop=(c == n_c - 1),
                    )
                cnt = rpool.tile([W_SEGS, 1], f32, tag="cnt")
                nc.vector.tensor_scalar(
                    out=cnt[:],
                    in0=ps[:, D : D + 1],
                    scalar1=1.0,
                    scalar2=None,
                    op0=mybir.AluOpType.max,
                )
                rec = rpool.tile([W_SEGS, 1], f32, tag="rec")
                nc.vector.reciprocal(out=rec[:], in_=cnt[:])
                ot = rpool.tile([W_SEGS, D], f32, tag="ot")
                nc.vector.tensor_scalar(
                    out=ot[:],
                    in0=ps[:, 0:D],
                    scalar1=rec[:, 0:1],
                    scalar2=None,
                    op0=mybir.AluOpType.mult,
                )
                nc.sync.dma_start(
                    out=outp[w * W_SEGS : (w + 1) * W_SEGS, :], in_=ot[:]
                )
    nc.compile()
    _PROGRAM_CACHE[key] = nc
    return nc


def _prepare_inputs(values, segment_ids, num_segments):
    E, d = values.shape
    assert d == D
    N = int(num_segments)
    sids = np.ascontiguousarray(np.asarray(segment_ids, dtype=np.int64))
    vals_f = np.ascontiguousarray(np.asarray(values), dtype=np.float32)

    segs_per_core = math.ceil(N / N_CORES)
    n_win = math.ceil(segs_per_core / W_SEGS)

    lows = []
    for k in range(N_CORES):
        for w in range(n_win):
            lo = min(N, k * segs_per_core + w * W_SEGS)
            hi = min(N, k * segs_per_core + min((w + 1) * W_SEGS, segs_per_core))
            lows.append((lo, hi))
    seg_bnds = np.asarray(lows, dtype=np.int64)
    e_lo = np.searchsorted(sids, seg_bnds[:, 0], side="left")
    e_hi = np.searchsorted(sids, seg_bnds[:, 1], side="left")
    n_edges = e_hi - e_lo
    n_c = max(1, int(np.max((n_edges + P - 1) // P)))
    slots = n_c * P

    in_maps = []
    for k in range(N_CORES):
        vals_k = np.zeros((n_win, slots, D + 1), dtype=np.float32)
        vals_k[:, :, D] = 1.0
        locs_k = np.full((n_win, slots), -1.0, dtype=np.float32)
        for w in range(n_win):
            i = k * n_win + w
            lo, hi = e_lo[i], e_hi[i]
            ne = int(hi - lo)
            if ne > 0:
                vals_k[w, :ne, :D] = vals_f[lo:hi]
                locs_k[w, :ne] = (sids[lo:hi] - seg_bnds[i, 0]).astype(np.float32)
        # [w, slot=(c p), e] -> [w, p, c, e] so each partition's DMA run is
        # one contiguous ~7KB stretch in DRAM
        vals_k = np.ascontiguousarray(
            vals_k.reshape(n_win, n_c, P, D + 1).transpose(0, 2, 1, 3)
        ).reshape(n_win, P, n_c * (D + 1))
        locs_k = np.ascontiguousarray(
            locs_k.reshape(n_win, n_c, P).transpose(0, 2, 1)
        )
        in_maps.append({"vals": vals_k, "locs": locs_k})
    return in_maps, n_win, n_c, segs_per_core, N


def kernel(values, segment_ids, num_segments):
    in_maps, n_win, n_c, segs_per_core, N = _prepare_inputs(
        values, segment_ids, num_segments
    )
    nc = _build_program(n_win, n_c)
    trace = bool(int(os.environ.get("KTRACE", "0")))
    res = run_bass_kernel_spmd(
        nc,
        in_maps,
        list(range(N_CORES)),
        trace=trace,
        tmpdir=os.environ.get("KTRACE_DIR") or None,
    )
    global LAST_RESULT
    LAST_RESULT = res
    parts = []
    for k in range(N_CORES):
        take = min(segs_per_core, N - k * segs_per_core)
        if take > 0:
            parts.append(res.results[k]["outp"][:take])
    return np.concatenate(parts, axis=0).astype(np.float32)


# revision 4
# speedup vs baseline: 1.3739x; 1.3739x over previous
"""Segment-mean (MeanAggregator) Trainium2 kernel.

Sorted segment ids -> shard *segments* evenly across the 8 cores (host
snaps edge ranges to segment boundaries) so each core owns a disjoint row
range of the output: no cross-core reduction.

Per core, segments are processed in uniform windows of 128 segments held
as a [128 segs, 65] PSUM accumulator (64 sums + count).  Edges stream in
chunks of 128 (the matmul contraction dim).  For each window, ONE
vector-engine tensor_tensor (with stride-0 broadcast APs) builds the
one-hot block O[edge, chunk, seg] = (loc[edge,chunk] == seg), and each
chunk does a single TensorE matmul psum += O_c.T @ [values | ones].
Padding slots carry loc = -1 whose one-hot row is all zero.

The per-window chunk count n_c[w] is the max over cores, so the program
is identical on all 8 cores (SPMD) while per-core data differs.
"""

import math
import os

import numpy as np

import concourse.bacc as bacc
import concourse.mybir as mybir
import concourse.tile as tile
from concourse.bass_utils import run_bass_kernel_spmd

N_CORES = 8
P = 128       # edges per chunk == contraction dim
W_SEGS = 128  # segments per PSUM window
D = 64

_PROGRAM_CACHE = {}


def _build_program(n_cs, vals_offs, locs_offs):
    """n_cs: chunk count per window; *_offs: flat element offsets per window."""
    key = (tuple(n_cs),)
    if key in _PROGRAM_CACHE:
        return _PROGRAM_CACHE[key]
    f32 = mybir.dt.float32
    n_win = len(n_cs)
    vals_total = vals_offs[-1]
    locs_total = locs_offs[-1]
    nc = bacc.Bacc(
        "TRN2", target_bir_lowering=False, debug=False, num_devices=N_CORES
    )
    vals = nc.dram_tensor("vals", [vals_total], f32, kind="ExternalInput")
    locs = nc.dram_tensor("locs", [locs_total], f32, kind="ExternalInput")
    outp = nc.dram_tensor("outp", [n_win * W_SEGS, D], f32, kind="ExternalOutput")

    with tile.TileContext(nc) as tc:
        with (
            tc.tile_pool(name="const", bufs=1) as cpool,
            tc.tile_pool(name="vals", bufs=3) as vpool,
            tc.tile_pool(name="locs", bufs=3) as lpool,
            tc.tile_pool(name="oh", bufs=2) as opool,
            tc.tile_pool(name="acc", bufs=2, space="PSUM") as ppool,
            tc.tile_pool(name="res", bufs=3) as rpool,
        ):
            iota = cpool.tile([P, W_SEGS], f32)
            nc.gpsimd.iota(
                iota[:],
                pattern=[[1, W_SEGS]],
                base=0,
                channel_multiplier=0,
                allow_small_or_imprecise_dtypes=True,
            )
            max_nc = max(n_cs)
            for w in range(n_win):
                n_c = n_cs[w]
                vt = vpool.tile([P, max_nc * (D + 1)], f32, tag="vt")
                nc.sync.dma_start(
                    out=vt[:, : n_c * (D + 1)],
                    in_=vals[vals_offs[w] : vals_offs[w + 1]].rearrange(
                        "(p f) -> p f", p=P
                    ),
                )
                vt3 = vt[:].rearrange("p (c e) -> p c e", e=D + 1)
                lt = lpool.tile([P, max_nc], f32, tag="lt")
                nc.sync.dma_start(
                    out=lt[:, :n_c],
                    in_=locs[locs_offs[w] : locs_offs[w + 1]].rearrange(
                        "(p f) -> p f", p=P
                    ),
                )
                oh = opool.tile([P, max_nc * W_SEGS], f32, tag="oh")
                oh3 = oh[:].rearrange("p (c j) -> p c j", j=W_SEGS)
                nc.vector.tensor_tensor(
                    out=oh3[:, :n_c, :],
                    in0=lt[:, :n_c, None].to_broadcast([P, n_c, W_SEGS]),
                    in1=iota[:, None, :].to_broadcast([P, n_c, W_SEGS]),
                    op=mybir.AluOpType.is_equal,
                )
                ps = ppool.tile([W_SEGS, D + 1], f32)
                for c in range(n_c):
                    nc.tensor.matmul(
                        out=ps[:],
                        lhsT=oh3[:, c, :],
                        rhs=vt3[:, c, :],
                        start=(c == 0),
                        stop=(c == n_c - 1),
                    )
                cnt = rpool.tile([W_SEGS, 1], f32, tag="cnt")
                nc.vector.tensor_scalar(
                    out=cnt[:],
                    in0=ps[:, D : D + 1],
                    scalar1=1.0,
                    scalar2=None,
                    op0=mybir.AluOpType.max,
                )
                rec = rpool.tile([W_SEGS, 1], f32, tag="rec")
                nc.vector.reciprocal(out=rec[:], in_=cnt[:])
                ot = rpool.tile([W_SEGS, D], f32, tag="ot")
                nc.vector.tensor_scalar(
                    out=ot[:],
                    in0=ps[:, 0:D],
                    scalar1=rec[:, 0:1],
                    scalar2=None,
                    op0=mybir.AluOpType.mult,
                )
                nc.sync.dma_start(
                    out=outp[w * W_SEGS : (w + 1) * W_SEGS, :], in_=ot[:]
                )
    nc.compile()
    _PROGRAM_CACHE[key] = nc
    return nc


def _prepare_inputs(values, segment_ids, num_segments):
    E, d = values.shape
    assert d == D
    N = int(num_segments)
    sids = np.ascontiguousarray(np.asarray(segment_ids, dtype=np.int64))
    vals_f = np.ascontiguousarray(np.asarray(values), dtype=np.float32)

    segs_per_core = math.ceil(N / N_CORES)
    n_win = math.ceil(segs_per_core / W_SEGS)

    bnds = []
    for k in range(N_CORES):
        for w in range(n_win):
            lo = min(N, k * segs_per_core + w * W_SEGS)
            hi = min(N, k * segs_per_core + min((w + 1) * W_SEGS, segs_per_core))
            bnds.append((lo, hi))
    seg_bnds = np.asarray(bnds, dtype=np.int64)
    e_lo = np.searchsorted(sids, seg_bnds[:, 0], side="left")
    e_hi = np.searchsorted(sids, seg_bnds[:, 1], side="left")
    n_edges = (e_hi - e_lo).reshape(N_CORES, n_win)
    n_cs = np.maximum(1, (n_edges.max(axis=0) + P - 1) // P).astype(int)  # [n_win]

    vals_offs = np.concatenate([[0], np.cumsum(n_cs * P * (D + 1))]).astype(int)
    locs_offs = np.concatenate([[0], np.cumsum(n_cs * P)]).astype(int)

    in_maps = []
    for k in range(N_CORES):
        vals_k = np.zeros(vals_offs[-1], dtype=np.float32)
        locs_k = np.full(locs_offs[-1], -1.0, dtype=np.float32)
        for w in range(n_win):
            i = k * n_win + w
            lo, hi = e_lo[i], e_hi[i]
            ne = int(hi - lo)
            n_c = n_cs[w]
            slots = n_c * P
            # window block: [slots, D+1] with ones col, then -> [P, n_c*(D+1)]
            blk = np.zeros((slots, D + 1), dtype=np.float32)
            blk[:, D] = 1.0
            if ne > 0:
                blk[:ne, :D] = vals_f[lo:hi]
            blk = blk.reshape(n_c, P, D + 1).transpose(1, 0, 2)
            vals_k[vals_offs[w] : vals_offs[w + 1]] = blk.reshape(-1)
            lblk = np.full(slots, -1.0, dtype=np.float32)
            if ne > 0:
                lblk[:ne] = (sids[lo:hi] - seg_bnds[i, 0]).astype(np.float32)
            lblk = lblk.reshape(n_c, P).transpose(1, 0)
            locs_k[locs_offs[w] : locs_offs[w + 1]] = lblk.reshape(-1)
        in_maps.append({"vals": vals_k, "locs": locs_k})
    return in_maps, list(n_cs), list(vals_offs), list(locs_offs), segs_per_core, N


def kernel(values, segment_ids, num_segments):
    in_maps, n_cs, vals_offs, locs_offs, segs_per_core, N = _prepare_inputs(
        values, segment_ids, num_segments
    )
    nc = _build_program(n_cs, vals_offs, locs_offs)
    trace = bool(int(os.environ.get("KTRACE", "0")))
    res = run_bass_kernel_spmd(
        nc,
        in_maps,
        list(range(N_CORES)),
        trace=trace,
        tmpdir=os.environ.get("KTRACE_DIR") or None,
    )
    global LAST_RESULT
    LAST_RESULT = res
    parts = []
    for k in range(N_CORES):
        take = min(segs_per_core, N - k * segs_per_core)
        if take > 0:
            parts.append(res.results[k]["outp"][:take])
    return np.concatenate(parts, axis=0).astype(np.float32)


# revision 5
# speedup vs baseline: 4.6149x; 3.3591x over previous
"""Segment-mean (MeanAggregator) Trainium2 kernel.

Sorted segment ids -> shard *segments* evenly across the 8 cores (host
snaps edge ranges to segment boundaries) so each core owns a disjoint row
range of the output: no cross-core reduction.

Per core, segments are processed in uniform windows of 128 segments held
as a [128 segs, 65] PSUM accumulator (64 sums + count).  Edges stream in
chunks of 128 (the matmul contraction dim).  For each window, ONE
vector-engine tensor_tensor (with stride-0 broadcast APs) builds the
one-hot block O[edge, chunk, seg] = (loc[edge,chunk] == seg), and each
chunk does TensorE matmul(s) psum += O_c.T @ [values | ones].  Padding
slots carry loc = -1 whose one-hot row is all zero.

MODE (dtype of the matmul operands):
  - "split": values split into bf16 hi + bf16 lo, both accumulated into
    the same fp32 PSUM (hi's ones-col carries the count, lo's is 0).
    Full-rate PE with FWL weight loads; ~1e-6 relative error.
  - "f16":   values cast to fp16 (halves the input DMA); ~3e-4 rel err.
  - "f32":   exact fp32 (PE runs 2 half-rate passes per matmul; slow).

The per-window chunk count n_c[w] is the max over cores, so the program
is identical on all 8 cores (SPMD) while per-core data differs.
"""

import math
import os

import ml_dtypes
import numpy as np

import concourse.bacc as bacc
import concourse.mybir as mybir
import concourse.tile as tile
from concourse.bass_utils import run_bass_kernel_spmd

N_CORES = 8
P = 128       # edges per chunk == contraction dim
W_SEGS = 128  # segments per PSUM window
D = 64

MODE = os.environ.get("KMODE", "split")

_PROGRAM_CACHE = {}


def _build_program(mode, n_cs, vals_offs, locs_offs):
    key = (mode, tuple(n_cs))
    if key in _PROGRAM_CACHE:
        return _PROGRAM_CACHE[key]
    f32 = mybir.dt.float32
    mmdt = {
        "split": mybir.dt.bfloat16,
        "f16": mybir.dt.float16,
        "f32": f32,
    }[mode]
    n_pass = 2 if mode == "split" else 1
    n_win = len(n_cs)
    nc = bacc.Bacc(
        "TRN2", target_bir_lowering=False, debug=False, num_devices=N_CORES
    )
    vals = nc.dram_tensor("vals", [vals_offs[-1]], mmdt, kind="ExternalInput")
    locs = nc.dram_tensor("locs", [locs_offs[-1]], f32, kind="ExternalInput")
    outp = nc.dram_tensor("outp", [n_win * W_SEGS, D], f32, kind="ExternalOutput")

    row = n_pass * (D + 1)  # columns per chunk in the vals tile

    with tile.TileContext(nc) as tc:
        with (
            tc.tile_pool(name="const", bufs=1) as cpool,
            tc.tile_pool(name="vals", bufs=3) as vpool,
            tc.tile_pool(name="locs", bufs=3) as lpool,
            tc.tile_pool(name="oh", bufs=3) as opool,
            tc.tile_pool(name="acc", bufs=2, space="PSUM") as ppool,
            tc.tile_pool(name="res", bufs=3) as rpool,
        ):
            iota = cpool.tile([P, W_SEGS], f32)
            nc.gpsimd.iota(
                iota[:],
                pattern=[[1, W_SEGS]],
                base=0,
                channel_multiplier=0,
                allow_small_or_imprecise_dtypes=True,
            )
            max_nc = max(n_cs)
            for w in range(n_win):
                n_c = n_cs[w]
                vt = vpool.tile([P, max_nc * row], mmdt, tag="vt")
                nc.sync.dma_start(
                    out=vt[:, : n_c * row],
                    in_=vals[vals_offs[w] : vals_offs[w + 1]].rearrange(
                        "(p f) -> p f", p=P
                    ),
                )
                vt4 = vt[:].rearrange("p (c s e) -> p c s e", s=n_pass, e=D + 1)
                lt = lpool.tile([P, max_nc], f32, tag="lt")
                nc.sync.dma_start(
                    out=lt[:, :n_c],
                    in_=locs[locs_offs[w] : locs_offs[w + 1]].rearrange(
                        "(p f) -> p f", p=P
                    ),
                )
                oh = opool.tile([P, max_nc * W_SEGS], mmdt, tag="oh")
                oh3 = oh[:].rearrange("p (c j) -> p c j", j=W_SEGS)
                nc.vector.tensor_tensor(
                    out=oh3[:, :n_c, :],
                    in0=lt[:, :n_c, None].to_broadcast([P, n_c, W_SEGS]),
                    in1=iota[:, None, :].to_broadcast([P, n_c, W_SEGS]),
                    op=mybir.AluOpType.is_equal,
                )
                ps = ppool.tile([W_SEGS, D + 1], f32)
                for c in range(n_c):
                    for s in range(n_pass):
                        nc.tensor.matmul(
                            out=ps[:],
                            lhsT=oh3[:, c, :],
                            rhs=vt4[:, c, s, :],
                            start=(c == 0 and s == 0),
                            stop=(c == n_c - 1 and s == n_pass - 1),
                        )
                cnt = rpool.tile([W_SEGS, 1], f32, tag="cnt")
                nc.vector.tensor_scalar(
                    out=cnt[:],
                    in0=ps[:, D : D + 1],
                    scalar1=1.0,
                    scalar2=None,
                    op0=mybir.AluOpType.max,
                )
                rec = rpool.tile([W_SEGS, 1], f32, tag="rec")
                nc.vector.reciprocal(out=rec[:], in_=cnt[:])
                ot = rpool.tile([W_SEGS, D], f32, tag="ot")
                nc.scalar.mul(out=ot[:], in_=ps[:, 0:D], mul=rec[:, 0:1])
                nc.sync.dma_start(
                    out=outp[w * W_SEGS : (w + 1) * W_SEGS, :], in_=ot[:]
                )
    nc.compile()
    _PROGRAM_CACHE[key] = nc
    return nc


def _prepare_inputs(values, segment_ids, num_segments, mode=MODE):
    E, d = values.shape
    assert d == D
    N = int(num_segments)
    sids = np.ascontiguousarray(np.asarray(segment_ids, dtype=np.int64))
    vals_f = np.ascontiguousarray(np.asarray(values), dtype=np.float32)

    n_pass = 2 if mode == "split" else 1
    np_mmdt = {
        "split": ml_dtypes.bfloat16,
        "f16": np.float16,
        "f32": np.float32,
    }[mode]
    row = n_pass * (D + 1)

    segs_per_core = math.ceil(N / N_CORES)
    n_win = math.ceil(segs_per_core / W_SEGS)

    bnds = []
    for k in range(N_CORES):
        for w in range(n_win):
            lo = min(N, k * segs_per_core + w * W_SEGS)
            hi = min(N, k * segs_per_core + min((w + 1) * W_SEGS, segs_per_core))
            bnds.append((lo, hi))
    seg_bnds = np.asarray(bnds, dtype=np.int64)
    e_lo = np.searchsorted(sids, seg_bnds[:, 0], side="left")
    e_hi = np.searchsorted(sids, seg_bnds[:, 1], side="left")
    n_edges = (e_hi - e_lo).reshape(N_CORES, n_win)
    n_cs = np.maximum(1, (n_edges.max(axis=0) + P - 1) // P).astype(int)

    vals_offs = np.concatenate([[0], np.cumsum(n_cs * P * row)]).astype(int)
    locs_offs = np.concatenate([[0], np.cumsum(n_cs * P)]).astype(int)

    in_maps = []
    for k in range(N_CORES):
        vals_k = np.zeros(vals_offs[-1], dtype=np_mmdt)
        locs_k = np.full(locs_offs[-1], -1.0, dtype=np.float32)
        for w in range(n_win):
            i = k * n_win + w
            lo, hi = e_lo[i], e_hi[i]
            ne = int(hi - lo)
            n_c = int(n_cs[w])
            slots = n_c * P
            blk = np.zeros((slots, n_pass, D + 1), dtype=np.float32)
            blk[:, 0, D] = 1.0
            if ne > 0:
                v = vals_f[lo:hi]
                if mode == "split":
                    hi16 = v.astype(ml_dtypes.bfloat16)
                    blk[:ne, 0, :D] = hi16
                    blk[:ne, 1, :D] = v - hi16.astype(np.float32)
                else:
                    blk[:ne, 0, :D] = v
            blk = blk.reshape(n_c, P, row).transpose(1, 0, 2)
            vals_k[vals_offs[w] : vals_offs[w + 1]] = (
                blk.astype(np_mmdt).reshape(-1)
            )
            lblk = np.full(slots, -1.0, dtype=np.float32)
            if ne > 0:
                lblk[:ne] = (sids[lo:hi] - seg_bnds[i, 0]).astype(np.float32)
            locs_k[locs_offs[w] : locs_offs[w + 1]] = (
                lblk.reshape(n_c, P).transpose(1, 0).reshape(-1)
            )
        in_maps.append({"vals": vals_k, "locs": locs_k})
    return in_maps, list(n_cs), list(vals_offs), list(locs_offs), segs_per_core, N


def kernel(values, segment_ids, num_segments):
    mode = MODE
    in_maps, n_cs, vals_offs, locs_offs, segs_per_core, N = _prepare_inputs(
        values, segment_ids, num_segments, mode
    )
    nc = _build_program(mode, n_cs, vals_offs, locs_offs)
    trace = bool(int(os.environ.get("KTRACE", "0")))
    res = run_bass_kernel_spmd(
        nc,
        in_maps,
        list(range(N_CORES)),
        trace=trace,
        tmpdir=os.environ.get("KTRACE_DIR") or None,
    )
    global LAST_RESULT
    LAST_RESULT = res
    parts = []
    for k in range(N_CORES):
        take = min(segs_per_core, N - k * segs_per_core)
        if take > 0:
            parts.append(res.results[k]["outp"][:take])
    return np.concatenate(parts, axis=0).astype(np.float32)
